# revision 22
# baseline (speedup 1.0000x reference)
"""Trainium2 Bass kernel for 2-layer GraphSAGE (mean aggregation) on 8 NeuronCores.

Math: with M = mean-aggregation operator (D^-1 A), the reference is
    h  = relu(x @ W1 + b1)
    h1 = (M h) Wl1 + bl1 + h Wr1
    h2 = (M h1) Wl2 + bl2 + h1 Wr2
    out = h2 @ W2 + b2
Everything after the relu is linear; by linearity of M the two aggregation
passes fold into 3-feature payloads each:
    out = M( M(h C2) + h C1 ) + h C0 + r*c_r + c_1
with C2 = Wl1 Wl2 W2, C1 = (Wr1 Wl2 + Wl1 Wr2) W2, C0 = Wr1 Wr2 W2,
c_r = bl1 Wl2 W2, c_1 = (bl2 + bl1 Wr2) W2 + b2, r = (deg > 0).
All constant folds (C*, c_*, 1/deg, r*c_r + c_1) are computed host-side.

Distribution: NC n owns src-shard n (12500 nodes; computes h locally).
Its incident edges are grouped by dst-range -> Q7 core, chunked and sorted
by dst.  Per chunk: GPSIMD ap_gather (feature-per-partition tables, int16
src-local indices) -> DVE cumulative-sum scan -> GPSIMD gather at segment
ends -> shifted subtract = per-dst sums.  The ends-gather of chunk k is
issued after the msg-gather of chunk k+1 so GPSIMD never waits on the DVE
scan.  A PE one-hot matmul compacts the (group, feature) partitions, and
partial sums are ReduceScattered across the 8 NCs (dst-shard n -> NC n),
combined with 1/deg and the local hC1 term on device, and fed to pass 2.
Host-side work is only integer graph partitioning / index layout.
"""
import numpy as np

# ---- problem constants (hardcoded per contract) ----
N = 100000
E = 6400000
IN_F = 128
HID = 10
OUT = 3

NCN = 8             # NeuronCores
QC = 8              # Q7 cores per NC
NSH = N // NCN      # 12500 nodes per shard
K_CH = 12           # chunks per (NC, q7core)
D_CHUNK = -(-NSH // K_CH)          # 1042 dsts per chunk
D_STRIP = K_CH * D_CHUNK           # 12504
NE = ((D_CHUNK + 1 + 15) // 16) * 16   # 1056 ends entries per chunk
NSH_TAB = ((NSH + 16 + 15) // 16) * 16  # 12528 table width
SENT = NSH + 6                      # sentinel (zero) table column
XPAD = ((NSH + 511) // 512) * 512   # 12800 padded x rows (512-row groups)
FB = D_STRIP // 8                   # 1563 cols per partition in 24-row layout
F32 = "float32"

_prog_cache = {}


def _build_program(C_CAP):
    from contextlib import ExitStack
    import concourse.bacc as bacc
    import concourse.tile as tile
    import concourse.mybir as mybir
    from concourse.masks import make_identity

    f32 = mybir.dt.float32
    i16 = mybir.dt.int16
    AL = mybir.AluOpType
    AF = mybir.ActivationFunctionType

    nc = bacc.Bacc("TRN2", target_bir_lowering=False, debug=False,
                   num_devices=NCN)

    # ---- I/O ----
    x_in = nc.dram_tensor("x_sh", [XPAD, IN_F], f32, kind="ExternalInput")
    eidx_in = nc.dram_tensor("eidx", [K_CH, 128, C_CAP // 16], i16, kind="ExternalInput")
    eend_in = nc.dram_tensor("eends", [K_CH, 128, NE // 16], i16, kind="ExternalInput")
    sel_in = nc.dram_tensor("sel24", [128, 24], f32, kind="ExternalInput")
    ccc_in = nc.dram_tensor("ccc", [HID, 9], f32, kind="ExternalInput")
    W1_in = nc.dram_tensor("W1", [IN_F, HID], f32, kind="ExternalInput")
    b1_in = nc.dram_tensor("b1c", [HID, 1], f32, kind="ExternalInput")
    recip3_in = nc.dram_tensor("recip3", [3, D_STRIP], f32, kind="ExternalInput")
    recip24_in = nc.dram_tensor("recip24", [24, FB], f32, kind="ExternalInput")
    bias24_in = nc.dram_tensor("bias24", [24, FB], f32, kind="ExternalInput")
    out_ext = nc.dram_tensor("outT", [24, FB], f32, kind="ExternalOutput")

    with tile.TileContext(nc) as tc:
        es = ExitStack()
        with es:
            dram = es.enter_context(tc.tile_pool(name="dram", bufs=1, space="DRAM"))
            p_small = es.enter_context(tc.tile_pool(name="small", bufs=1))

            bounceA = dram.tile([NCN, 3, D_STRIP], f32)
            bounceC = dram.tile([NCN, 3, D_STRIP], f32)
            rsA = dram.tile([3, D_STRIP], f32)
            rsC = dram.tile([3, D_STRIP], f32)
            gc19_d = dram.tile([6, XPAD], f32)

            sel = p_small.tile([128, 24], f32)
            nc.sync.dma_start(out=sel[:], in_=sel_in[:])
            w1 = p_small.tile([IN_F, HID], f32)
            nc.sync.dma_start(out=w1[:], in_=W1_in[:])
            ccc = p_small.tile([HID, 9], f32)
            nc.sync.dma_start(out=ccc[:], in_=ccc_in[:])
            b1c = p_small.tile([HID, 1], f32)
            nc.sync.dma_start(out=b1c[:], in_=b1_in[:])
            r24 = p_small.tile([24, FB], f32)
            nc.scalar.dma_start(out=r24[:], in_=recip24_in[:])
            sb24 = p_small.tile([24, FB], f32)
            nc.scalar.dma_start(out=sb24[:], in_=bias24_in[:])

            # ---- phase 1: h = relu(x W1 + b1); gcf = [hC2 | hC1 | hC0] ----
            p_tab = es.enter_context(tc.tile_pool(name="tab", bufs=1))
            tab = p_tab.tile([128, NSH_TAB], f32)
            nc.scalar.memzero(tab[:])
            with tc.tile_pool(name="lin1", bufs=3) as p_lin, \
                 tc.tile_pool(name="gcf", bufs=1) as p_gcf, \
                 tc.tile_pool(name="lin1ps", bufs=2, space="PSUM") as p_lps:
                gcf = p_gcf.tile([9, XPAD], f32)
                ident = p_small.tile([128, 128], f32)
                make_identity(nc, ident[:])
                n_grp = XPAD // 512
                for g in range(n_grp):
                    xt4 = p_lin.tile([128, 4, 128], f32, tag="xt4")
                    # rows 512g..512g+512 of x -> [p, t, f]
                    eng = nc.sync if g % 2 == 0 else nc.scalar
                    eng.dma_start(
                        out=xt4[:],
                        in_=x_in[:].rearrange("(a t p) f -> a p t f", a=XPAD // 512, t=4, p=128)[g])
                    tps = p_lps.tile([128, 512], f32, space="PSUM", tag="tps")
                    for t in range(4):
                        nc.tensor.transpose(out=tps[:, t * 128:(t + 1) * 128],
                                            in_=xt4[:, t, :], identity=ident[:])
                    xtb = p_lin.tile([128, 512], f32, tag="xtb")
                    nc.vector.tensor_copy(out=xtb[:], in_=tps[:])
                    hps = p_lps.tile([HID, 512], f32, space="PSUM", tag="hps")
                    nc.tensor.matmul(out=hps[:], lhsT=w1[:], rhs=xtb[:], start=True, stop=True)
                    hb = p_lin.tile([HID, 512], f32, tag="hb")
                    nc.scalar.activation(out=hb[:], in_=hps[:], func=AF.Relu,
                                         bias=b1c[:], scale=1.0)
                    gps = p_lps.tile([9, 512], f32, space="PSUM", tag="gps")
                    nc.tensor.matmul(out=gps[:], lhsT=ccc[:], rhs=hb[:], start=True, stop=True)
                    nc.vector.tensor_copy(out=gcf[:, g * 512:(g + 1) * 512], in_=gps[:])
                # zero the padded-node columns so they never pollute tables
                nc.vector.memset(gcf[:, D_STRIP:XPAD], 0.0)
                # distribute hC2 into the gather table (3 rows per 16-row group)
                for g in range(QC):
                    eng = nc.sync if g % 2 == 0 else nc.scalar
                    eng.dma_start(out=tab[16 * g:16 * g + 3, 0:D_STRIP],
                                  in_=gcf[0:3, 0:D_STRIP])
                # stash hC1 | hC0 for the inter-pass/final stages
                nc.sync.dma_start(out=gc19_d[:], in_=gcf[3:9, :])

            # whole-pass gather index tables, loaded once and shared by both
            # passes: removes all per-chunk DMA triggers/waits from the
            # GPSIMD critical path
            p_idx = es.enter_context(tc.tile_pool(name="agg_idx", bufs=1))
            idx_tiles, end_tiles = [], []
            for k in range(K_CH):
                eng = nc.sync if k % 2 == 0 else nc.scalar
                it = p_idx.tile([128, C_CAP // 16], i16, tag=f"i{k}")
                eng.dma_start(out=it[:], in_=eidx_in[k])
                idx_tiles.append(it)
                et = p_idx.tile([128, NE // 16], i16, tag=f"e{k}")
                eng.dma_start(out=et[:], in_=eend_in[k])
                end_tiles.append(et)

            # ---- aggregation passes ----
            def agg_pass(view24):
                with tc.tile_pool(name="agg_msg", bufs=2) as p_msg, \
                     tc.tile_pool(name="agg_sm", bufs=4) as p_asm, \
                     tc.tile_pool(name="agg_ps", bufs=2, space="PSUM") as p_aps:
                    def finish(st):
                        msg, k = st
                        gat = p_asm.tile([128, NE], f32, tag="gat")
                        nc.gpsimd.ap_gather(
                            out_ap=gat[:], in_ap=msg[:], idxs_ap=end_tiles[k][:],
                            channels=128, num_elems=C_CAP, d=1, num_idxs=NE)
                        strip = p_asm.tile([128, D_CHUNK], f32, tag="strip")
                        nc.vector.tensor_tensor(
                            out=strip[:], in0=gat[:, 1:1 + D_CHUNK],
                            in1=gat[:, 0:D_CHUNK], op=AL.subtract)
                        comp = p_asm.tile([24, D_CHUNK], f32, tag="comp")
                        for j in range(0, D_CHUNK, 512):
                            w = min(512, D_CHUNK - j)
                            cps = p_aps.tile([24, w], f32, space="PSUM", tag="cps")
                            nc.tensor.matmul(out=cps[:], lhsT=sel[:],
                                             rhs=strip[:, j:j + w], start=True, stop=True)
                            nc.vector.tensor_copy(out=comp[:, j:j + w], in_=cps[:])
                        nc.sync.dma_start(
                            out=view24[:, k * D_CHUNK:(k + 1) * D_CHUNK],
                            in_=comp[:])

                    state = None
                    for k in range(K_CH):
                        msg = p_msg.tile([128, C_CAP], f32, tag="msg")
                        nc.gpsimd.ap_gather(
                            out_ap=msg[:], in_ap=tab[:], idxs_ap=idx_tiles[k][:],
                            channels=128, num_elems=NSH_TAB, d=1, num_idxs=C_CAP)
                        if state is not None:
                            finish(state)
                        nc.vector.tensor_tensor_scan(
                            out=msg[:], data0=msg[:], data1=msg[:], initial=0.0,
                            op0=AL.add, op1=AL.bypass)
                        state = (msg, k)
                    finish(state)

            vA = bounceA[:].rearrange("g f d -> (g f) d")
            vC = bounceC[:].rearrange("g f d -> (g f) d")
            rg = [list(range(NCN))]

            agg_pass(vA)
            nc.gpsimd.collective_compute("ReduceScatter", AL.add, replica_groups=rg,
                                         ins=[bounceA.opt()], outs=[rsA.opt()])

            # pass-2 table: t2 = rsA * recip + hC1, written into the same rows
            with tc.tile_pool(name="mid", bufs=2) as p_mid:
                ta = p_mid.tile([3, D_STRIP], f32, tag="wide")
                nc.sync.dma_start(out=ta[:], in_=rsA[:])
                td = p_mid.tile([3, D_STRIP], f32, tag="wide")
                nc.scalar.dma_start(out=td[:], in_=recip3_in[:])
                nc.vector.tensor_tensor(out=ta[:], in0=ta[:], in1=td[:], op=AL.mult)
                tg = p_mid.tile([3, D_STRIP], f32, tag="wide")
                nc.scalar.dma_start(out=tg[:], in_=gc19_d[0:3, 0:D_STRIP])
                nc.vector.tensor_tensor(out=ta[:], in0=ta[:], in1=tg[:], op=AL.add)
                for g in range(QC):
                    eng = nc.sync if g % 2 == 0 else nc.scalar
                    eng.dma_start(out=tab[16 * g:16 * g + 3, 0:D_STRIP], in_=ta[:])

            agg_pass(vC)
            nc.gpsimd.collective_compute("ReduceScatter", AL.add, replica_groups=rg,
                                         ins=[bounceC.opt()], outs=[rsC.opt()])

            # ---- final: out = rsC*recip + hC0 + (r*c_r + c_1) ----
            # 24-row layout: partition b*3+f holds feature f of dsts [b*FB,(b+1)*FB)
            with tc.tile_pool(name="fin", bufs=2) as p_fin:
                s1 = p_fin.tile([24, FB], f32, tag="fw")
                sg = p_fin.tile([24, FB], f32, tag="fw")
                for b in range(8):
                    eng = nc.sync if b % 2 == 0 else nc.scalar
                    eng.dma_start(out=s1[3 * b:3 * b + 3, :],
                                  in_=rsC[:, b * FB:(b + 1) * FB])
                    eng.dma_start(out=sg[3 * b:3 * b + 3, :],
                                  in_=gc19_d[3:6, b * FB:(b + 1) * FB])
                nc.vector.tensor_tensor(out=s1[:], in0=s1[:], in1=r24[:], op=AL.mult)
                nc.vector.tensor_tensor(out=s1[:], in0=s1[:], in1=sg[:], op=AL.add)
                nc.vector.tensor_tensor(out=s1[:], in0=s1[:], in1=sb24[:], op=AL.add)
                nc.sync.dma_start(out=out_ext[:], in_=s1[:])

    nc.compile()
    return nc


def _balance_chunks(deg_nd):
    """Assign each shard's local dsts to chunks, balancing per-NC cell loads.

    Returns (K_of, POS_of, dst_of_strip): chunk id / in-chunk position per
    global dst, and per-shard strip-position -> local-dst map (-1 = pad)."""
    K_of = np.empty(N, np.int64)
    POS_of = np.empty(N, np.int64)
    dst_of_strip = np.full((NCN, D_STRIP), -1, np.int64)
    for c in range(NCN):
        degs = deg_nd[:, c * NSH:(c + 1) * NSH]          # [NCN, NSH]
        order = np.argsort(-degs.sum(axis=0), kind="stable")
        load = np.zeros((NCN, K_CH))
        cnt = np.zeros(K_CH, np.int64)
        kk = np.empty(NSH, np.int64)
        pos = np.empty(NSH, np.int64)
        degs_o = degs[:, order]
        for i in range(NSH):
            nm = np.max(load + degs_o[:, i][:, None], axis=0)
            nm[cnt >= D_CHUNK] = np.inf
            k = int(np.argmin(nm))
            d = order[i]
            kk[d] = k
            pos[d] = cnt[k]
            load[:, k] += degs_o[:, i]
            cnt[k] += 1
        K_of[c * NSH:(c + 1) * NSH] = kk
        POS_of[c * NSH:(c + 1) * NSH] = pos
        dst_of_strip[c, kk * D_CHUNK + pos] = np.arange(NSH)
    return K_of, POS_of, dst_of_strip


def _preprocess(inputs):
    """Pure-integer graph partitioning + host constant folds; returns in_maps."""
    x = np.ascontiguousarray(np.asarray(inputs["x"], dtype=np.float32))
    edge_index = np.asarray(inputs["edge_index"])
    src = edge_index[0].astype(np.int64)
    dst = edge_index[1].astype(np.int64)

    n_of_e = src // NSH
    c_of_e = dst // NSH

    # Balanced chunk assignment, applied as a within-shard node RELABELING:
    # node's new local id == its strip position, so every device-side array
    # (x shard, tables, gcf, recip/bias strips) stays aligned by construction.
    deg_nd = np.bincount(n_of_e * np.int64(N) + dst, minlength=NCN * N)
    K_of, POS_of, dst_of_strip = _balance_chunks(deg_nd.reshape(NCN, N))
    strip_of = K_of * np.int64(D_CHUNK) + POS_of   # new local id per global node
    src_loc = strip_of[src].astype(np.int16)
    k_of_e = K_of[dst]
    d_of_e = POS_of[dst]

    key = (((n_of_e * QC + c_of_e) * K_CH + k_of_e) * np.int64(D_CHUNK)
           + d_of_e)
    order = np.argsort(key, kind="stable")
    so = src_loc[order]
    no = n_of_e[order]
    co = c_of_e[order]
    ko = k_of_e[order]
    do_ = d_of_e[order]

    cell = (no * QC + co) * K_CH + ko
    n_cells = NCN * QC * K_CH
    cnt = np.bincount(cell, minlength=n_cells)
    C_CAP = ((int(cnt.max()) + 1 + 15) // 16) * 16
    cell_start = np.zeros(n_cells + 1, np.int64)
    np.cumsum(cnt, out=cell_start[1:])
    slot = np.arange(E) - cell_start[cell] + 1        # slot 0 = sentinel pad

    eidx = np.full((NCN, K_CH, 128, C_CAP // 16), SENT, np.int16)
    eidx[no, ko, 16 * co + (slot % 16), slot // 16] = so

    dcell = cell * D_CHUNK + do_
    cnt_d = np.bincount(dcell, minlength=n_cells * D_CHUNK).reshape(
        NCN, QC, K_CH, D_CHUNK)
    ce = np.cumsum(cnt_d, axis=3).astype(np.int16)
    ends_ext = np.zeros((NCN, QC, K_CH, NE), np.int16)
    ends_ext[:, :, :, 1:D_CHUNK + 1] = ce
    ends_ext[:, :, :, D_CHUNK + 1:] = ce[:, :, :, -1:]
    eends = np.zeros((NCN, K_CH, 128, NE // 16), np.int16)
    for c in range(QC):
        w = ends_ext[:, c].reshape(NCN, K_CH, NE // 16, 16)
        eends[:, :, 16 * c:16 * c + 16, :] = np.swapaxes(w, 2, 3)

    # host-side constant folds
    W1 = np.ascontiguousarray(np.asarray(inputs["W1"], np.float32))
    b1 = np.asarray(inputs["b1"], np.float32)
    Wl1 = np.asarray(inputs["Wl1"], np.float32)
    bl1 = np.asarray(inputs["bl1"], np.float32)
    Wr1 = np.asarray(inputs["Wr1"], np.float32)
    Wl2 = np.asarray(inputs["Wl2"], np.float32)
    bl2 = np.asarray(inputs["bl2"], np.float32)
    Wr2 = np.asarray(inputs["Wr2"], np.float32)
    W2 = np.asarray(inputs["W2"], np.float32)
    b2 = np.asarray(inputs["b2"], np.float32)

    C2 = Wl1 @ Wl2 @ W2
    C1 = (Wr1 @ Wl2 + Wl1 @ Wr2) @ W2
    C0 = Wr1 @ Wr2 @ W2
    c_r = bl1 @ Wl2 @ W2
    c_1 = (bl2 + bl1 @ Wr2) @ W2 + b2
    ccc = np.ascontiguousarray(
        np.concatenate([C2, C1, C0], axis=1).astype(np.float32))  # [10, 9]

    deg = np.bincount(dst, minlength=N).astype(np.float32)
    recip_all = 1.0 / np.maximum(deg, 1.0)
    r_all = (deg > 0).astype(np.float32)

    sel = np.zeros((128, 24), np.float32)
    for g in range(QC):
        for f in range(3):
            sel[16 * g + f, 3 * g + f] = 1.0

    in_maps = []
    for n in range(NCN):
        xs = np.zeros((XPAD, IN_F), np.float32)
        xs[strip_of[n * NSH:(n + 1) * NSH]] = x[n * NSH:(n + 1) * NSH]
        # shard-local per-dst constants in (balanced) strip order
        sv = dst_of_strip[n]
        valid = sv >= 0
        rs = np.zeros(D_STRIP, np.float32)
        rs[valid] = recip_all[n * NSH + sv[valid]]
        rr = np.zeros(D_STRIP, np.float32)
        rr[valid] = r_all[n * NSH + sv[valid]]
        recip3 = np.ascontiguousarray(np.broadcast_to(rs, (3, D_STRIP)))
        # 24-row layout [b*3+f, j] <-> feature f of dst-local b*FB+j
        r24 = np.ascontiguousarray(
            np.broadcast_to(rs.reshape(8, 1, FB), (8, 3, FB)).reshape(24, FB))
        bias24 = np.ascontiguousarray(
            (rr.reshape(8, 1, FB) * c_r.reshape(1, 3, 1)
             + c_1.reshape(1, 3, 1)).reshape(24, FB).astype(np.float32))
        m = {
            "x_sh": xs,
            "eidx": np.ascontiguousarray(eidx[n]),
            "eends": np.ascontiguousarray(eends[n]),
            "sel24": sel,
            "ccc": ccc,
            "W1": W1,
            "b1c": np.ascontiguousarray(b1.reshape(HID, 1)),
            "recip3": recip3,
            "recip24": r24,
            "bias24": bias24,
        }
        in_maps.append(m)
    return in_maps, C_CAP, dst_of_strip


def kernel(**inputs) -> np.ndarray:
    from concourse.bass_utils import run_bass_kernel_spmd

    in_maps, C_CAP, dst_of_strip = _preprocess(inputs)
    if C_CAP not in _prog_cache:
        _prog_cache[C_CAP] = _build_program(C_CAP)
    nc = _prog_cache[C_CAP]
    res = run_bass_kernel_spmd(nc, in_maps, core_ids=list(range(NCN)))
    out = np.empty((N, OUT), np.float32)
    for n in range(NCN):
        o = res.results[n]["outT"].reshape(8, 3, FB)
        rows = o.transpose(0, 2, 1).reshape(D_STRIP, 3)
        sv = dst_of_strip[n]
        valid = sv >= 0
        out[n * NSH + sv[valid]] = rows[valid]
    return out



# revision 23
# speedup vs baseline: 1.2049x; 1.2049x over previous
"""Trainium2 Bass kernel for 2-layer GraphSAGE (mean aggregation) on 8 NeuronCores.

Math: with M = mean-aggregation operator (D^-1 A), the reference is
    h  = relu(x @ W1 + b1)
    h1 = (M h) Wl1 + bl1 + h Wr1
    h2 = (M h1) Wl2 + bl2 + h1 Wr2
    out = h2 @ W2 + b2
Everything after the relu is linear; by linearity of M the two aggregation
passes fold into 3-feature payloads each:
    out = M( M(h C2) + h C1 ) + h C0 + r*c_r + c_1
with C2 = Wl1 Wl2 W2, C1 = (Wr1 Wl2 + Wl1 Wr2) W2, C0 = Wr1 Wr2 W2,
c_r = bl1 Wl2 W2, c_1 = (bl2 + bl1 Wr2) W2 + b2, r = (deg > 0).
All constant folds (C*, c_*, 1/deg, r*c_r + c_1) are computed host-side.

Distribution: NC n owns src-shard n (12500 nodes; computes h locally).
Its incident edges are grouped by dst-range -> Q7 core, chunked and sorted
by dst.  Per chunk: GPSIMD ap_gather (feature-per-partition tables, int16
src-local indices) -> DVE cumulative-sum scan -> GPSIMD gather at segment
ends -> shifted subtract = per-dst sums.  The ends-gather of chunk k is
issued after the msg-gather of chunk k+1 so GPSIMD never waits on the DVE
scan.  A PE one-hot matmul compacts the (group, feature) partitions, and
partial sums are ReduceScattered across the 8 NCs (dst-shard n -> NC n),
combined with 1/deg and the local hC1 term on device, and fed to pass 2.
Host-side work is only integer graph partitioning / index layout.
"""
import numpy as np

# ---- problem constants (hardcoded per contract) ----
N = 100000
E = 6400000
IN_F = 128
HID = 10
OUT = 3

NCN = 8             # NeuronCores
QC = 8              # Q7 cores per NC
NSH = N // NCN      # 12500 nodes per shard
K_CH = 12           # chunks per (NC, q7core)
D_CHUNK = -(-NSH // K_CH)          # 1042 dsts per chunk
D_STRIP = K_CH * D_CHUNK           # 12504
NE = ((D_CHUNK + 1 + 15) // 16) * 16   # 1056 ends entries per chunk
NSH_TAB = ((NSH + 16 + 15) // 16) * 16  # 12528 table width
SENT = NSH + 6                      # sentinel (zero) table column
XPAD = ((NSH + 511) // 512) * 512   # 12800 padded x rows (512-row groups)
FB = D_STRIP // 8                   # 1563 cols per partition in 24-row layout
F32 = "float32"

_prog_cache = {}


def _build_program(C_CAP):
    from contextlib import ExitStack
    import concourse.bacc as bacc
    import concourse.tile as tile
    import concourse.mybir as mybir
    from concourse.masks import make_identity

    f32 = mybir.dt.float32
    i16 = mybir.dt.int16
    AL = mybir.AluOpType
    AF = mybir.ActivationFunctionType

    nc = bacc.Bacc("TRN2", target_bir_lowering=False, debug=False,
                   num_devices=NCN)

    # ---- I/O ----
    x_in = nc.dram_tensor("x_sh", [XPAD, IN_F], f32, kind="ExternalInput")
    eidx_in = nc.dram_tensor("eidx", [K_CH, 128, C_CAP // 16], i16, kind="ExternalInput")
    eend_in = nc.dram_tensor("eends", [K_CH, 128, NE // 16], i16, kind="ExternalInput")
    sel_in = nc.dram_tensor("sel24", [128, 24], f32, kind="ExternalInput")
    ccc_in = nc.dram_tensor("ccc", [HID, 9], f32, kind="ExternalInput")
    W1_in = nc.dram_tensor("W1", [IN_F, HID], f32, kind="ExternalInput")
    b1_in = nc.dram_tensor("b1c", [HID, 1], f32, kind="ExternalInput")
    recip3_in = nc.dram_tensor("recip3", [3, D_STRIP], f32, kind="ExternalInput")
    recip24_in = nc.dram_tensor("recip24", [24, FB], f32, kind="ExternalInput")
    bias24_in = nc.dram_tensor("bias24", [24, FB], f32, kind="ExternalInput")
    out_ext = nc.dram_tensor("outT", [24, FB], f32, kind="ExternalOutput")

    with tile.TileContext(nc) as tc:
        es = ExitStack()
        with es:
            dram = es.enter_context(tc.tile_pool(name="dram", bufs=1, space="DRAM"))
            p_small = es.enter_context(tc.tile_pool(name="small", bufs=1))

            bounceA = dram.tile([NCN, 3, D_STRIP], f32)
            bounceC = dram.tile([NCN, 3, D_STRIP], f32)
            rsA = dram.tile([3, D_STRIP], f32)
            rsC = dram.tile([3, D_STRIP], f32)
            gc19_d = dram.tile([6, XPAD], f32)

            sel = p_small.tile([128, 24], f32)
            nc.sync.dma_start(out=sel[:], in_=sel_in[:])
            w1 = p_small.tile([IN_F, HID], f32)
            nc.sync.dma_start(out=w1[:], in_=W1_in[:])
            ccc = p_small.tile([HID, 9], f32)
            nc.sync.dma_start(out=ccc[:], in_=ccc_in[:])
            b1c = p_small.tile([HID, 1], f32)
            nc.sync.dma_start(out=b1c[:], in_=b1_in[:])
            r24 = p_small.tile([24, FB], f32)
            nc.scalar.dma_start(out=r24[:], in_=recip24_in[:])
            sb24 = p_small.tile([24, FB], f32)
            nc.scalar.dma_start(out=sb24[:], in_=bias24_in[:])

            # ---- phase 1: h = relu(x W1 + b1); gcf = [hC2 | hC1 | hC0] ----
            p_tab = es.enter_context(tc.tile_pool(name="tab", bufs=1))
            tab = p_tab.tile([128, NSH_TAB], f32)
            nc.scalar.memzero(tab[:])
            with tc.tile_pool(name="lin1", bufs=3) as p_lin, \
                 tc.tile_pool(name="gcf", bufs=1) as p_gcf, \
                 tc.tile_pool(name="lin1ps", bufs=2, space="PSUM") as p_lps:
                gcf = p_gcf.tile([9, XPAD], f32)
                ident = p_small.tile([128, 128], f32)
                make_identity(nc, ident[:])
                n_grp = XPAD // 512
                for g in range(n_grp):
                    xt4 = p_lin.tile([128, 4, 128], f32, tag="xt4")
                    # rows 512g..512g+512 of x -> [p, t, f]
                    eng = nc.sync if g % 2 == 0 else nc.scalar
                    eng.dma_start(
                        out=xt4[:],
                        in_=x_in[:].rearrange("(a t p) f -> a p t f", a=XPAD // 512, t=4, p=128)[g])
                    tps = p_lps.tile([128, 512], f32, space="PSUM", tag="tps")
                    for t in range(4):
                        nc.tensor.transpose(out=tps[:, t * 128:(t + 1) * 128],
                                            in_=xt4[:, t, :], identity=ident[:])
                    xtb = p_lin.tile([128, 512], f32, tag="xtb")
                    nc.vector.tensor_copy(out=xtb[:], in_=tps[:])
                    hps = p_lps.tile([HID, 512], f32, space="PSUM", tag="hps")
                    nc.tensor.matmul(out=hps[:], lhsT=w1[:], rhs=xtb[:], start=True, stop=True)
                    hb = p_lin.tile([HID, 512], f32, tag="hb")
                    nc.scalar.activation(out=hb[:], in_=hps[:], func=AF.Relu,
                                         bias=b1c[:], scale=1.0)
                    gps = p_lps.tile([9, 512], f32, space="PSUM", tag="gps")
                    nc.tensor.matmul(out=gps[:], lhsT=ccc[:], rhs=hb[:], start=True, stop=True)
                    nc.vector.tensor_copy(out=gcf[:, g * 512:(g + 1) * 512], in_=gps[:])
                # zero the padded-node columns so they never pollute tables
                nc.vector.memset(gcf[:, D_STRIP:XPAD], 0.0)
                # distribute hC2 into the gather table (3 rows per 16-row group)
                for g in range(QC):
                    eng = nc.sync if g % 2 == 0 else nc.scalar
                    eng.dma_start(out=tab[16 * g:16 * g + 3, 0:D_STRIP],
                                  in_=gcf[0:3, 0:D_STRIP])
                # stash hC1 | hC0 for the inter-pass/final stages
                nc.sync.dma_start(out=gc19_d[:], in_=gcf[3:9, :])

            # ---- aggregation passes ----
            def agg_pass(view24):
                with tc.tile_pool(name="agg_msg", bufs=2) as p_msg, \
                     tc.tile_pool(name="agg_sm", bufs=4) as p_asm, \
                     tc.tile_pool(name="agg_ps", bufs=2, space="PSUM") as p_aps:
                    def finish(st):
                        msg, end_t, k = st
                        gat = p_asm.tile([128, NE], f32, tag="gat")
                        nc.gpsimd.ap_gather(
                            out_ap=gat[:], in_ap=msg[:], idxs_ap=end_t[:],
                            channels=128, num_elems=C_CAP, d=1, num_idxs=NE)
                        strip = p_asm.tile([128, D_CHUNK], f32, tag="strip")
                        nc.vector.tensor_tensor(
                            out=strip[:], in0=gat[:, 1:1 + D_CHUNK],
                            in1=gat[:, 0:D_CHUNK], op=AL.subtract)
                        comp = p_asm.tile([24, D_CHUNK], f32, tag="comp")
                        for j in range(0, D_CHUNK, 512):
                            w = min(512, D_CHUNK - j)
                            cps = p_aps.tile([24, w], f32, space="PSUM", tag="cps")
                            nc.tensor.matmul(out=cps[:], lhsT=sel[:],
                                             rhs=strip[:, j:j + w], start=True, stop=True)
                            nc.vector.tensor_copy(out=comp[:, j:j + w], in_=cps[:])
                        nc.sync.dma_start(
                            out=view24[:, k * D_CHUNK:(k + 1) * D_CHUNK],
                            in_=comp[:])

                    state = None
                    for k in range(K_CH):
                        idx_t = p_asm.tile([128, C_CAP // 16], i16, tag="idx")
                        nc.sync.dma_start(out=idx_t[:], in_=eidx_in[k])
                        end_t = p_asm.tile([128, NE // 16], i16, tag="end")
                        nc.sync.dma_start(out=end_t[:], in_=eend_in[k])
                        msg = p_msg.tile([128, C_CAP], f32, tag="msg")
                        nc.gpsimd.ap_gather(
                            out_ap=msg[:], in_ap=tab[:], idxs_ap=idx_t[:],
                            channels=128, num_elems=NSH_TAB, d=1, num_idxs=C_CAP)
                        if state is not None:
                            finish(state)
                        nc.vector.tensor_tensor_scan(
                            out=msg[:], data0=msg[:], data1=msg[:], initial=0.0,
                            op0=AL.add, op1=AL.bypass)
                        state = (msg, end_t, k)
                    finish(state)

            vA = bounceA[:].rearrange("g f d -> (g f) d")
            vC = bounceC[:].rearrange("g f d -> (g f) d")
            rg = [list(range(NCN))]

            agg_pass(vA)
            nc.gpsimd.collective_compute("ReduceScatter", AL.add, replica_groups=rg,
                                         ins=[bounceA.opt()], outs=[rsA.opt()])

            # pass-2 table: t2 = rsA * recip + hC1, written into the same rows
            with tc.tile_pool(name="mid", bufs=2) as p_mid:
                ta = p_mid.tile([3, D_STRIP], f32, tag="wide")
                nc.sync.dma_start(out=ta[:], in_=rsA[:])
                td = p_mid.tile([3, D_STRIP], f32, tag="wide")
                nc.scalar.dma_start(out=td[:], in_=recip3_in[:])
                nc.vector.tensor_tensor(out=ta[:], in0=ta[:], in1=td[:], op=AL.mult)
                tg = p_mid.tile([3, D_STRIP], f32, tag="wide")
                nc.scalar.dma_start(out=tg[:], in_=gc19_d[0:3, 0:D_STRIP])
                nc.vector.tensor_tensor(out=ta[:], in0=ta[:], in1=tg[:], op=AL.add)
                for g in range(QC):
                    eng = nc.sync if g % 2 == 0 else nc.scalar
                    eng.dma_start(out=tab[16 * g:16 * g + 3, 0:D_STRIP], in_=ta[:])

            agg_pass(vC)
            nc.gpsimd.collective_compute("ReduceScatter", AL.add, replica_groups=rg,
                                         ins=[bounceC.opt()], outs=[rsC.opt()])

            # ---- final: out = rsC*recip + hC0 + (r*c_r + c_1) ----
            # 24-row layout: partition b*3+f holds feature f of dsts [b*FB,(b+1)*FB)
            with tc.tile_pool(name="fin", bufs=2) as p_fin:
                s1 = p_fin.tile([24, FB], f32, tag="fw")
                sg = p_fin.tile([24, FB], f32, tag="fw")
                for b in range(8):
                    eng = nc.sync if b % 2 == 0 else nc.scalar
                    eng.dma_start(out=s1[3 * b:3 * b + 3, :],
                                  in_=rsC[:, b * FB:(b + 1) * FB])
                    eng.dma_start(out=sg[3 * b:3 * b + 3, :],
                                  in_=gc19_d[3:6, b * FB:(b + 1) * FB])
                nc.vector.tensor_tensor(out=s1[:], in0=s1[:], in1=r24[:], op=AL.mult)
                nc.vector.tensor_tensor(out=s1[:], in0=s1[:], in1=sg[:], op=AL.add)
                nc.vector.tensor_tensor(out=s1[:], in0=s1[:], in1=sb24[:], op=AL.add)
                nc.sync.dma_start(out=out_ext[:], in_=s1[:])

    nc.compile()
    return nc


def _balance_chunks(deg_nd):
    """Assign each shard's local dsts to chunks, balancing per-NC cell loads.

    Returns (K_of, POS_of, dst_of_strip): chunk id / in-chunk position per
    global dst, and per-shard strip-position -> local-dst map (-1 = pad)."""
    K_of = np.empty(N, np.int64)
    POS_of = np.empty(N, np.int64)
    dst_of_strip = np.full((NCN, D_STRIP), -1, np.int64)
    for c in range(NCN):
        degs = deg_nd[:, c * NSH:(c + 1) * NSH]          # [NCN, NSH]
        order = np.argsort(-degs.sum(axis=0), kind="stable")
        load = np.zeros((NCN, K_CH))
        cnt = np.zeros(K_CH, np.int64)
        kk = np.empty(NSH, np.int64)
        pos = np.empty(NSH, np.int64)
        degs_o = degs[:, order]
        for i in range(NSH):
            nm = np.max(load + degs_o[:, i][:, None], axis=0)
            nm[cnt >= D_CHUNK] = np.inf
            k = int(np.argmin(nm))
            d = order[i]
            kk[d] = k
            pos[d] = cnt[k]
            load[:, k] += degs_o[:, i]
            cnt[k] += 1
        K_of[c * NSH:(c + 1) * NSH] = kk
        POS_of[c * NSH:(c + 1) * NSH] = pos
        dst_of_strip[c, kk * D_CHUNK + pos] = np.arange(NSH)
    return K_of, POS_of, dst_of_strip


def _preprocess(inputs):
    """Pure-integer graph partitioning + host constant folds; returns in_maps."""
    x = np.ascontiguousarray(np.asarray(inputs["x"], dtype=np.float32))
    edge_index = np.asarray(inputs["edge_index"])
    src = edge_index[0].astype(np.int64)
    dst = edge_index[1].astype(np.int64)

    n_of_e = src // NSH
    c_of_e = dst // NSH

    # Balanced chunk assignment, applied as a within-shard node RELABELING:
    # node's new local id == its strip position, so every device-side array
    # (x shard, tables, gcf, recip/bias strips) stays aligned by construction.
    deg_nd = np.bincount(n_of_e * np.int64(N) + dst, minlength=NCN * N)
    K_of, POS_of, dst_of_strip = _balance_chunks(deg_nd.reshape(NCN, N))
    strip_of = K_of * np.int64(D_CHUNK) + POS_of   # new local id per global node
    src_loc = strip_of[src].astype(np.int16)
    k_of_e = K_of[dst]
    d_of_e = POS_of[dst]

    key = (((n_of_e * QC + c_of_e) * K_CH + k_of_e) * np.int64(D_CHUNK)
           + d_of_e)
    order = np.argsort(key, kind="stable")
    so = src_loc[order]
    no = n_of_e[order]
    co = c_of_e[order]
    ko = k_of_e[order]
    do_ = d_of_e[order]

    cell = (no * QC + co) * K_CH + ko
    n_cells = NCN * QC * K_CH
    cnt = np.bincount(cell, minlength=n_cells)
    C_CAP = ((int(cnt.max()) + 1 + 15) // 16) * 16
    cell_start = np.zeros(n_cells + 1, np.int64)
    np.cumsum(cnt, out=cell_start[1:])
    slot = np.arange(E) - cell_start[cell] + 1        # slot 0 = sentinel pad

    eidx = np.full((NCN, K_CH, 128, C_CAP // 16), SENT, np.int16)
    eidx[no, ko, 16 * co + (slot % 16), slot // 16] = so

    dcell = cell * D_CHUNK + do_
    cnt_d = np.bincount(dcell, minlength=n_cells * D_CHUNK).reshape(
        NCN, QC, K_CH, D_CHUNK)
    ce = np.cumsum(cnt_d, axis=3).astype(np.int16)
    ends_ext = np.zeros((NCN, QC, K_CH, NE), np.int16)
    ends_ext[:, :, :, 1:D_CHUNK + 1] = ce
    ends_ext[:, :, :, D_CHUNK + 1:] = ce[:, :, :, -1:]
    eends = np.zeros((NCN, K_CH, 128, NE // 16), np.int16)
    for c in range(QC):
        w = ends_ext[:, c].reshape(NCN, K_CH, NE // 16, 16)
        eends[:, :, 16 * c:16 * c + 16, :] = np.swapaxes(w, 2, 3)

    # host-side constant folds
    W1 = np.ascontiguousarray(np.asarray(inputs["W1"], np.float32))
    b1 = np.asarray(inputs["b1"], np.float32)
    Wl1 = np.asarray(inputs["Wl1"], np.float32)
    bl1 = np.asarray(inputs["bl1"], np.float32)
    Wr1 = np.asarray(inputs["Wr1"], np.float32)
    Wl2 = np.asarray(inputs["Wl2"], np.float32)
    bl2 = np.asarray(inputs["bl2"], np.float32)
    Wr2 = np.asarray(inputs["Wr2"], np.float32)
    W2 = np.asarray(inputs["W2"], np.float32)
    b2 = np.asarray(inputs["b2"], np.float32)

    C2 = Wl1 @ Wl2 @ W2
    C1 = (Wr1 @ Wl2 + Wl1 @ Wr2) @ W2
    C0 = Wr1 @ Wr2 @ W2
    c_r = bl1 @ Wl2 @ W2
    c_1 = (bl2 + bl1 @ Wr2) @ W2 + b2
    ccc = np.ascontiguousarray(
        np.concatenate([C2, C1, C0], axis=1).astype(np.float32))  # [10, 9]

    deg = np.bincount(dst, minlength=N).astype(np.float32)
    recip_all = 1.0 / np.maximum(deg, 1.0)
    r_all = (deg > 0).astype(np.float32)

    sel = np.zeros((128, 24), np.float32)
    for g in range(QC):
        for f in range(3):
            sel[16 * g + f, 3 * g + f] = 1.0

    in_maps = []
    for n in range(NCN):
        xs = np.zeros((XPAD, IN_F), np.float32)
        xs[strip_of[n * NSH:(n + 1) * NSH]] = x[n * NSH:(n + 1) * NSH]
        # shard-local per-dst constants in (balanced) strip order
        sv = dst_of_strip[n]
        valid = sv >= 0
        rs = np.zeros(D_STRIP, np.float32)
        rs[valid] = recip_all[n * NSH + sv[valid]]
        rr = np.zeros(D_STRIP, np.float32)
        rr[valid] = r_all[n * NSH + sv[valid]]
        recip3 = np.ascontiguousarray(np.broadcast_to(rs, (3, D_STRIP)))
        # 24-row layout [b*3+f, j] <-> feature f of dst-local b*FB+j
        r24 = np.ascontiguousarray(
            np.broadcast_to(rs.reshape(8, 1, FB), (8, 3, FB)).reshape(24, FB))
        bias24 = np.ascontiguousarray(
            (rr.reshape(8, 1, FB) * c_r.reshape(1, 3, 1)
             + c_1.reshape(1, 3, 1)).reshape(24, FB).astype(np.float32))
        m = {
            "x_sh": xs,
            "eidx": np.ascontiguousarray(eidx[n]),
            "eends": np.ascontiguousarray(eends[n]),
            "sel24": sel,
            "ccc": ccc,
            "W1": W1,
            "b1c": np.ascontiguousarray(b1.reshape(HID, 1)),
            "recip3": recip3,
            "recip24": r24,
            "bias24": bias24,
        }
        in_maps.append(m)
    return in_maps, C_CAP, dst_of_strip


def kernel(**inputs) -> np.ndarray:
    from concourse.bass_utils import run_bass_kernel_spmd

    in_maps, C_CAP, dst_of_strip = _preprocess(inputs)
    if C_CAP not in _prog_cache:
        _prog_cache[C_CAP] = _build_program(C_CAP)
    nc = _prog_cache[C_CAP]
    res = run_bass_kernel_spmd(nc, in_maps, core_ids=list(range(NCN)))
    out = np.empty((N, OUT), np.float32)
    for n in range(NCN):
        o = res.results[n]["outT"].reshape(8, 3, FB)
        rows = o.transpose(0, 2, 1).reshape(D_STRIP, 3)
        sv = dst_of_strip[n]
        valid = sv >= 0
        out[n * NSH + sv[valid]] = rows[valid]
    return out

